# revision 5
# baseline (speedup 1.0000x reference)
"""Trainium2 Bass kernel for nn_Block (ragged transformer block).

B=2, T=2048, D=768, H=12, DH=64, FF=3072.

Sharding: 8 cores = 2 batches x 4 query-blocks of 512 tokens.
Each core computes K^T/V over the full sequence of its batch (replicated
within the 4-core batch group), and Q/attention/proj/LN/FFN for its own
512 tokens. No collectives; the host gathers the 8 disjoint output slabs.

On-chip layout is feature-major ("transposed", [feature, token]) end to
end, so every matmul chains with no transposes:
  xT -> Q^T/K^T (feat-major) and V (token-major) -> S^T = K^T.T @ Q^T
  -> exp (key mask folded into the ACT bias) -> U^T = V'.T @ expS^T
  (softmax sums via ones-bias columns baked into V') -> a^T -> proj
  -> LN1 (stats via ones-matmul, broadcast for free) -> FFN (gelu bias
  on ACT) -> LN2 -> h^T.

All matmuls run in float32r (full PE rate at N>=256, ~1.6e-4 rel err).
Padded query rows are zeroed via a host-provided row mask, matching the
reference exactly (its LN outputs are masked the same way).
"""
import sys
for _p in ("/opt/trn_rl_repo", "/root/.axon_site/_ro/trn_rl_repo"):
    if _p not in sys.path:
        sys.path.append(_p)

from contextlib import ExitStack
import numpy as np

B, T, D, H, DH, FF = 2, 2048, 768, 12, 64, 3072
M = 512            # tokens per core
DC = 6             # D / 128
FC = 24            # FF / 128
NKC = 16           # T / 128
VW = H * (DH + 1)  # 780: V' width (64 cols + 1 ones-bias col per head)
EPS = 1e-5
NEG = -1.0e9

_STATE: dict = {}


def _build_program():
    import concourse.mybir as mybir
    import concourse.tile as tile
    from concourse import bacc

    F32 = mybir.dt.float32
    F32R = mybir.dt.float32r
    AF = mybir.ActivationFunctionType
    OP = mybir.AluOpType

    nc = bacc.Bacc("TRN2", target_bir_lowering=False, debug=False, num_devices=8)

    def din(name, shape, dt=F32R):
        return nc.dram_tensor(name, shape, dt, kind="ExternalInput").ap()

    xT = din("xT", [D, T])
    xTq = din("xTq", [D, M])
    wq = din("wq", [D, D])
    bq = din("bq", [1, D])
    wk = din("wk", [D, D])
    bk = din("bk", [1, D])
    wv = din("wv", [D, VW])
    bv = din("bv", [1, VW])
    wproj = din("wproj", [D, D])
    wfc = din("wfc", [D, FF])
    wout = din("wout", [FF, D])
    onesr = din("onesr", [1, M])
    ones128 = din("ones128", [128, 128])
    bprj = din("bprj", [128, DC], F32)
    bfc = din("bfc", [128, FC], F32)
    bout = din("bout", [128, DC], F32)
    l1g = din("l1g", [128, DC], F32)
    l1b = din("l1b", [128, DC], F32)
    l2g = din("l2g", [128, DC], F32)
    l2b = din("l2b", [128, DC], F32)
    kbias = din("kbias", [128, NKC], F32)
    epsc = din("epsc", [128, 1], F32)
    rowmask = din("rowmask", [128, M], F32)

    hT = nc.dram_tensor("hT", [D, M], F32, kind="ExternalOutput").ap()

    xT_r = xT.rearrange("(c p) n -> p c n", p=128)
    xTq_r = xTq.rearrange("(c p) n -> p c n", p=128)
    wq_r = wq.rearrange("(c p) n -> p c n", p=128)
    wk_r = wk.rearrange("(c p) n -> p c n", p=128)
    wv_r = wv.rearrange("(c p) n -> p c n", p=128)
    wproj_r = wproj.rearrange("(c p) n -> p c n", p=128)
    wfc_r = wfc.rearrange("(c p) n -> p c n", p=128)
    hT_r = hT.rearrange("(c p) n -> c p n", p=128)

    with tile.TileContext(nc) as tc, ExitStack() as ctx:
        const = ctx.enter_context(tc.tile_pool(name="const", bufs=1))

        def cload(name, shape, dt, src):
            t = const.tile(shape, dt, tag=name, name=name + "_t")
            nc.sync.dma_start(out=t, in_=src)
            return t

        onesr_t = cload("onesr", [1, M], F32R, onesr)
        ones128_t = cload("ones128", [128, 128], F32R, ones128)
        kbias_t = cload("kbias", [128, NKC], F32, kbias)
        epsc_t = cload("epsc", [128, 1], F32, epsc)
        rowmask_t = cload("rowmask", [128, M], F32, rowmask)
        bprj_t = cload("bprj", [128, DC], F32, bprj)
        bfc_t = cload("bfc", [128, FC], F32, bfc)
        bout_t = cload("bout", [128, DC], F32, bout)
        l1g_t = cload("l1g", [128, DC], F32, l1g)
        l1b_t = cload("l1b", [128, DC], F32, l1b)
        l2g_t = cload("l2g", [128, DC], F32, l2g)
        l2b_t = cload("l2b", [128, DC], F32, l2b)

        # ---------------- LN helper (transposed layout) ----------------
        def ln_apply(y_t, psum_sum, psum_ssq, g_t, b_t, out_cb):
            with tc.tile_pool(name="lnp", bufs=1) as lnp, \
                 tc.tile_pool(name="lns", bufs=2) as lns:
                m_bc = lnp.tile([128, M], F32, tag="mbc", name="mbc")
                nc.vector.tensor_scalar_mul(m_bc, psum_sum, 1.0 / D)
                mm = lnp.tile([128, M], F32, tag="mm", name="mm")
                nc.vector.tensor_mul(mm, m_bc, m_bc)
                var = lnp.tile([128, M], F32, tag="var", name="var")
                nc.vector.scalar_tensor_tensor(var, psum_ssq, 1.0 / D, mm,
                                               op0=OP.mult, op1=OP.subtract)
                sd = lnp.tile([128, M], F32, tag="sd", name="sd")
                nc.scalar.activation(sd, var, AF.Sqrt, bias=epsc_t[:, 0:1])
                rstd = lnp.tile([128, M], F32, tag="rstd", name="rstd")
                nc.vector.reciprocal(rstd, sd)
                for c2 in range(DC):
                    t1 = lns.tile([128, M], F32, tag="t1", name="t1")
                    nc.vector.tensor_sub(t1, y_t[:, c2, :].bitcast(F32), m_bc)
                    t2 = lns.tile([128, M], F32, tag="t2", name="t2")
                    nc.vector.tensor_mul(t2, t1, rstd)
                    t3 = lns.tile([128, M], F32, tag="t3", name="t3")
                    nc.vector.tensor_scalar(t3, t2, g_t[:, c2:c2 + 1],
                                            b_t[:, c2:c2 + 1],
                                            op0=OP.mult, op1=OP.add)
                    out_cb(c2, t3)

        big = ctx.enter_context(tc.tile_pool(name="big", bufs=1))
        if True:
            kT_t = big.tile([128, DC, T], F32R, tag="slotL", name="kT")
            vP_t = big.tile([128, NKC, VW], F32R, tag="slotM", name="vP")
            qT_t = big.tile([128, DC, M], F32R, tag="slotS", name="qT")

            # ---------------- Phase 1: QKV projections ----------------
            with tc.tile_pool(name="p1", bufs=1) as p1, \
                 tc.tile_pool(name="p1s", bufs=2) as p1s, \
                 tc.tile_pool(name="p1x", bufs=1) as p1x, \
                 tc.tile_pool(name="ps1", bufs=3, space="PSUM") as ps1, \
                 tc.tile_pool(name="ps1b", bufs=2, space="PSUM") as ps1b:
                bq_t = p1.tile([1, D], F32R, tag="bq", name="bq_t")
                nc.sync.dma_start(out=bq_t, in_=bq)
                bk_t = p1.tile([1, D], F32R, tag="bk", name="bk_t")
                nc.sync.dma_start(out=bk_t, in_=bk)
                bv_t = p1.tile([1, VW], F32R, tag="bv", name="bv_t")
                nc.sync.dma_start(out=bv_t, in_=bv)
                wv_t = p1.tile([128, DC, VW], F32R, tag="wv", name="wv_t")
                nc.sync.dma_start(out=wv_t, in_=wv_r)
                xTq_t = p1.tile([128, DC, M], F32R, tag="xTq", name="xTq_t")
                nc.sync.dma_start(out=xTq_t, in_=xTq_r)

                # Q^T [768, 512]
                for qc in range(DC):
                    wq_c = p1s.tile([128, DC, 128], F32R, tag="wqs", name="wq_c")
                    nc.sync.dma_start(out=wq_c,
                                      in_=wq_r[:, :, qc * 128:(qc + 1) * 128])
                    pq = ps1.tile([128, M], F32, tag="pq", name="pq")
                    for dc in range(DC):
                        nc.tensor.matmul(pq, wq_c[:, dc, :], xTq_t[:, dc, :],
                                         start=(dc == 0), stop=False)
                    nc.tensor.matmul(pq, bq_t[0:1, qc * 128:(qc + 1) * 128],
                                     onesr_t, start=False, stop=True)
                    nc.vector.tensor_copy(qT_t[:, qc, :], pq)

                # K^T [768, 2048] and V' [2048, 780] per token-block
                for tb in range(4):
                    xtb = p1x.tile([128, DC, M], F32R, tag="xtb", name="xtb")
                    nc.sync.dma_start(out=xtb, in_=xT_r[:, :, tb * M:(tb + 1) * M])
                    for kc in range(DC):
                        wk_c = p1s.tile([128, DC, 128], F32R, tag="wks", name="wk_c")
                        nc.sync.dma_start(out=wk_c,
                                          in_=wk_r[:, :, kc * 128:(kc + 1) * 128])
                        pk = ps1.tile([128, M], F32, tag="pk", name="pk")
                        for dc in range(DC):
                            nc.tensor.matmul(pk, wk_c[:, dc, :], xtb[:, dc, :],
                                             start=(dc == 0), stop=False)
                        nc.tensor.matmul(pk, bk_t[0:1, kc * 128:(kc + 1) * 128],
                                         onesr_t, start=False, stop=True)
                        nc.vector.tensor_copy(kT_t[:, kc, tb * M:(tb + 1) * M], pk)
                    for tq in range(4):
                        tci = tb * 4 + tq
                        for vb in range(2):
                            pv = ps1b.tile([128, VW // 2], F32, tag="pv", name="pv")
                            for dc in range(DC):
                                nc.tensor.matmul(
                                    pv, xtb[:, dc, tq * 128:(tq + 1) * 128],
                                    wv_t[:, dc, vb * (VW // 2):(vb + 1) * (VW // 2)],
                                    start=(dc == 0), stop=False)
                            nc.tensor.matmul(
                                pv, onesr_t[0:1, 0:128],
                                bv_t[0:1, vb * (VW // 2):(vb + 1) * (VW // 2)],
                                start=False, stop=True)
                            nc.vector.tensor_copy(
                                vP_t[:, tci, vb * (VW // 2):(vb + 1) * (VW // 2)],
                                pv)

            # ---------------- Phase 2: attention ----------------
            if True:
                aT_t = big.tile([128, DC, M], F32R, tag="slotA", name="aT")
                with tc.tile_pool(name="att", bufs=3) as attp, \
                     tc.tile_pool(name="atts", bufs=2) as atts, \
                     tc.tile_pool(name="psS", bufs=3, space="PSUM") as psS, \
                     tc.tile_pool(name="psU", bufs=2, space="PSUM") as psU, \
                     tc.tile_pool(name="psB", bufs=2, space="PSUM") as psB:
                    for h in range(H):
                        po = (h % 2) * 64
                        chk = h // 2
                        pu = psU.tile([128, M], F32, tag="pu", name="pu")
                        for kc in range(NKC):
                            s = psS.tile([128, M], F32, tag="s", name="s")
                            nc.tensor.matmul(
                                s, kT_t[po:po + 64, chk, kc * 128:(kc + 1) * 128],
                                qT_t[po:po + 64, chk, :], start=True, stop=True)
                            e = attp.tile([128, M], F32R, tag="exp", name="e")
                            nc.scalar.activation(e, s, AF.Exp,
                                                 bias=kbias_t[:, kc:kc + 1])
                            nc.tensor.matmul(
                                pu[0:DH + 1, :],
                                vP_t[:, kc, h * (DH + 1):(h + 1) * (DH + 1)], e,
                                start=(kc == 0), stop=(kc == NKC - 1))
                        srow = atts.tile([1, M], F32R, tag="srow", name="srow")
                        nc.vector.tensor_copy(srow, pu[DH:DH + 1, :])
                        pb = psB.tile([64, M], F32, tag="pb", name="pb")
                        nc.tensor.matmul(pb, onesr_t[0:1, 0:64], srow,
                                         start=True, stop=True)
                        rinv = atts.tile([64, M], F32, tag="rinv", name="rinv")
                        nc.vector.reciprocal(rinv, pb)
                        nc.vector.tensor_mul(aT_t[po:po + 64, chk, :],
                                             pu[0:64, :], rinv)

                # ------------ Phase 3: proj + residual + LN1 ------------
                if True:
                    nT_t = big.tile([128, DC, M], F32R, tag="slotS", name="nT")
                    with tc.tile_pool(name="p3", bufs=1) as p3, \
                         tc.tile_pool(name="p3s", bufs=2) as p3s, \
                         tc.tile_pool(name="psP", bufs=2, space="PSUM") as psP, \
                         tc.tile_pool(name="psT", bufs=1, space="PSUM") as psT:
                        wproj_t = p3.tile([128, DC, D], F32R, tag="wproj",
                                          name="wproj_t")
                        nc.sync.dma_start(out=wproj_t, in_=wproj_r)
                        y1_t = big.tile([128, DC, M], F32R, tag="slotL",
                                        name="y1")
                        psum_sum = psT.tile([128, M], F32, tag="s1",
                                            name="psum_sum")
                        psum_ssq = psT.tile([128, M], F32, tag="s2",
                                            name="psum_ssq")
                        for do in range(DC):
                            pp = psP.tile([128, M], F32, tag="pp", name="pp")
                            for di in range(DC):
                                nc.tensor.matmul(
                                    pp, wproj_t[:, di, do * 128:(do + 1) * 128],
                                    aT_t[:, di, :], start=(di == 0),
                                    stop=(di == DC - 1))
                            xr = p3s.tile([128, M], F32R, tag="xr", name="xr")
                            nc.sync.dma_start(out=xr, in_=xTq_r[:, do, :])
                            nc.vector.scalar_tensor_tensor(
                                y1_t[:, do, :], pp, bprj_t[:, do:do + 1],
                                xr.bitcast(F32), op0=OP.add, op1=OP.add)
                            sq = p3s.tile([128, M], F32R, tag="sq", name="sq")
                            nc.vector.tensor_mul(sq, y1_t[:, do, :].bitcast(F32),
                                                 y1_t[:, do, :].bitcast(F32))
                            nc.tensor.matmul(psum_sum, ones128_t, y1_t[:, do, :],
                                             start=(do == 0), stop=(do == DC - 1))
                            nc.tensor.matmul(psum_ssq, ones128_t, sq,
                                             start=(do == 0), stop=(do == DC - 1))

                        def to_nT(c2, t3):
                            nc.vector.tensor_mul(nT_t[:, c2, :], t3, rowmask_t)

                        ln_apply(y1_t, psum_sum, psum_ssq, l1g_t, l1b_t, to_nT)

                    # ------------ Phase 4: FFN + residual + LN2 ------------
                    with tc.tile_pool(name="p4a", bufs=3) as p4a, \
                         tc.tile_pool(name="p4h", bufs=2) as p4h, \
                         tc.tile_pool(name="psM", bufs=1, space="PSUM") as psM:
                        psm = [psM.tile([128, M], F32, tag=f"m{do}",
                                        name=f"psm{do}") for do in range(DC)]
                        with tc.tile_pool(name="p4w", bufs=3) as p4w, \
                             tc.tile_pool(name="psF", bufs=2, space="PSUM") as psF:
                            for f in range(FC):
                                wfcf = p4w.tile([128, DC, 128], F32R, tag="wfcf",
                                                name="wfcf")
                                nc.sync.dma_start(
                                    out=wfcf,
                                    in_=wfc_r[:, :, f * 128:(f + 1) * 128])
                                woutf = p4w.tile([128, D], F32R, tag="woutf",
                                                 name="woutf")
                                nc.sync.dma_start(
                                    out=woutf, in_=wout[f * 128:(f + 1) * 128, :])
                                pf = psF.tile([128, M], F32, tag="pf", name="pf")
                                for dc in range(DC):
                                    nc.tensor.matmul(pf, wfcf[:, dc, :],
                                                     nT_t[:, dc, :],
                                                     start=(dc == 0),
                                                     stop=(dc == DC - 1))
                                a1 = p4a.tile([128, M], F32R, tag="a1", name="a1")
                                nc.scalar.activation(a1, pf, AF.Gelu_apprx_tanh,
                                                     bias=bfc_t[:, f:f + 1])
                                for do in range(DC):
                                    nc.tensor.matmul(
                                        psm[do],
                                        woutf[:, do * 128:(do + 1) * 128],
                                        a1, start=(f == 0), stop=(f == FC - 1))

                        with tc.tile_pool(name="psT2", bufs=1,
                                          space="PSUM") as psT2:
                            y2_t = big.tile([128, DC, M], F32R,
                                            tag="slotM", name="y2")
                            psum_sum2 = psT2.tile([128, M], F32, tag="s1",
                                                  name="psum_sum2")
                            psum_ssq2 = psT2.tile([128, M], F32, tag="s2",
                                                  name="psum_ssq2")
                            for do in range(DC):
                                nc.vector.scalar_tensor_tensor(
                                    y2_t[:, do, :], psm[do],
                                    bout_t[:, do:do + 1],
                                    nT_t[:, do, :].bitcast(F32),
                                    op0=OP.add, op1=OP.add)
                                sq = p4a.tile([128, M], F32R, tag="sq2",
                                              name="sq2")
                                nc.vector.tensor_mul(
                                    sq, y2_t[:, do, :].bitcast(F32),
                                    y2_t[:, do, :].bitcast(F32))
                                nc.tensor.matmul(psum_sum2, ones128_t,
                                                 y2_t[:, do, :],
                                                 start=(do == 0),
                                                 stop=(do == DC - 1))
                                nc.tensor.matmul(psum_ssq2, ones128_t, sq,
                                                 start=(do == 0),
                                                 stop=(do == DC - 1))

                            def to_h(c2, t3):
                                hc = p4h.tile([128, M], F32, tag="hc", name="hc")
                                nc.vector.tensor_mul(hc, t3, rowmask_t)
                                nc.sync.dma_start(out=hT_r[c2], in_=hc)

                            ln_apply(y2_t, psum_sum2, psum_ssq2, l2g_t, l2b_t,
                                     to_h)

    nc.compile()
    return nc


def _shared_arrays(inputs):
    f32 = np.float32
    w_qkv = np.ascontiguousarray(inputs["w_qkv"], dtype=f32)
    b_qkv = np.ascontiguousarray(inputs["b_qkv"], dtype=f32)

    def pc(v):  # [C*128] -> [128, C] column-chunk layout
        v = np.ascontiguousarray(v, dtype=f32)
        return np.ascontiguousarray(v.reshape(-1, 128).T)

    wv_ext = np.zeros((D, VW), f32)
    bv_ext = np.zeros((1, VW), f32)
    for h in range(H):
        wv_ext[:, h * (DH + 1):h * (DH + 1) + DH] = \
            w_qkv[:, 2 * D + h * DH:2 * D + (h + 1) * DH]
        bv_ext[0, h * (DH + 1):h * (DH + 1) + DH] = \
            b_qkv[2 * D + h * DH:2 * D + (h + 1) * DH]
        bv_ext[0, h * (DH + 1) + DH] = 1.0

    return dict(
        wq=np.ascontiguousarray(w_qkv[:, 0:D]),
        bq=np.ascontiguousarray(b_qkv[0:D])[None, :],
        wk=np.ascontiguousarray(w_qkv[:, D:2 * D]),
        bk=np.ascontiguousarray(b_qkv[D:2 * D])[None, :],
        wv=wv_ext,
        bv=bv_ext,
        wproj=np.ascontiguousarray(inputs["w_proj"], dtype=f32),
        wfc=np.ascontiguousarray(inputs["w_fc"], dtype=f32),
        wout=np.ascontiguousarray(inputs["w_out"], dtype=f32),
        onesr=np.ones((1, M), f32),
        epsc=np.full((128, 1), EPS, f32),
        ones128=np.ones((128, 128), f32),
        bprj=pc(inputs["b_proj"]),
        bfc=pc(inputs["b_fc"]),
        bout=pc(inputs["b_out"]),
        l1g=pc(inputs["ln1_g"]),
        l1b=pc(inputs["ln1_b"]),
        l2g=pc(inputs["ln2_g"]),
        l2b=pc(inputs["ln2_b"]),
    )


def make_in_maps(inputs):
    inputs = {k: np.asarray(v) for k, v in inputs.items()}
    x = np.ascontiguousarray(inputs["x"], dtype=np.float32)
    lengths = np.asarray(inputs["lengths"]).astype(np.int64)
    shared = _shared_arrays(inputs)
    pos = np.arange(T)
    in_maps = []
    for c in range(8):
        b, r = divmod(c, 4)
        sl = slice(r * M, (r + 1) * M)
        xTb = np.ascontiguousarray(x[b].T)
        kb = np.where(pos < lengths[b], 0.0, NEG).astype(np.float32)
        rm = (pos[sl] < lengths[b]).astype(np.float32)
        m = dict(shared)
        m["xT"] = xTb
        m["xTq"] = np.ascontiguousarray(xTb[:, sl])
        m["kbias"] = np.ascontiguousarray(kb.reshape(NKC, 128).T)
        m["rowmask"] = np.ascontiguousarray(np.broadcast_to(rm[None, :], (128, M)))
        in_maps.append(m)
    return in_maps


def get_program():
    if "nc" not in _STATE:
        _STATE["nc"] = _build_program()
    return _STATE["nc"]


def kernel(**inputs) -> np.ndarray:
    from concourse.bass_utils import run_bass_kernel_spmd

    nc = get_program()
    in_maps = make_in_maps(inputs)
    res = run_bass_kernel_spmd(nc, in_maps, list(range(8)), trace=False)
    out = np.zeros((B, T, D), np.float32)
    for c in range(8):
        b, r = divmod(c, 4)
        out[b, r * M:(r + 1) * M, :] = res.results[c]["hT"].T
    return out


# revision 21
# speedup vs baseline: 1.4638x; 1.4638x over previous
"""Trainium2 Bass kernel for nn_Block (ragged transformer block).

B=2, T=2048, D=768, H=12, DH=64, FF=3072.

Sharding: 8 cores = 2 batches x 4 query-blocks of 512 tokens.
Each core computes K^T/V over the full sequence of its batch (replicated
within the 4-core batch group), and Q/attention/proj/LN/FFN for its own
512 tokens. No collectives; the host gathers the 8 disjoint output slabs.

On-chip layout is feature-major ("transposed", [feature, token]) end to
end, so every matmul chains with no transposes:
  xT -> Q^T/K^T (feat-major) and V (token-major) -> S^T = K^T.T @ Q^T
  -> exp (key mask folded into the ACT bias) -> U^T = V'.T @ expS^T
  (softmax sums via ones-bias columns baked into V') -> a^T -> proj
  -> LN1 (stats via ones-matmul, broadcast for free) -> FFN (gelu bias
  on ACT) -> LN2 -> h^T.

All matmuls run in float32r (full PE rate at N>=256, ~1.6e-4 rel err).
Padded query rows are zeroed via a host-provided row mask, matching the
reference exactly (its LN outputs are masked the same way).
"""
import sys
for _p in ("/opt/trn_rl_repo", "/root/.axon_site/_ro/trn_rl_repo"):
    if _p not in sys.path:
        sys.path.append(_p)

from contextlib import ExitStack
import numpy as np

B, T, D, H, DH, FF = 2, 2048, 768, 12, 64, 3072
M = 512            # tokens per core
DC = 6             # D / 128
FC = 24            # FF / 128
NKC = 16           # T / 128
VW = H * (DH + 1)  # 780: V' width (64 cols + 1 ones-bias col per head)
EPS = 1e-5
NEG = -1.0e9

_STATE: dict = {}


def _build_program(reps=1):
    import concourse.mybir as mybir
    import concourse.tile as tile
    from concourse import bacc

    F32 = mybir.dt.float32
    F32R = mybir.dt.float32r
    AF = mybir.ActivationFunctionType
    OP = mybir.AluOpType

    nc = bacc.Bacc("TRN2", target_bir_lowering=False, debug=False, num_devices=8)

    def din(name, shape, dt=F32R):
        return nc.dram_tensor(name, shape, dt, kind="ExternalInput").ap()

    xT = din("xT", [D, T])
    xTq = din("xTq", [D, M])
    wq = din("wq", [D, D])
    wk = din("wk", [D, D])
    bq_pc = din("bq_pc", [128, DC], F32)
    bk_pc = din("bk_pc", [128, DC], F32)
    wv = din("wv", [D, VW])
    bv = din("bv", [1, VW])
    wproj = din("wproj", [D, D])
    wfc = din("wfc", [D, FF])
    wt1 = din("wt1", [1, FF])
    wt2 = din("wt2", [1, FF])
    wout = din("wout", [FF, D])
    onesr = din("onesr", [1, M])
    ones128 = din("ones128", [128, 128])
    bprj = din("bprj", [128, DC], F32)
    bfc = din("bfc", [128, FC], F32)
    bout = din("bout", [128, DC], F32)
    l1g = din("l1g", [128, DC], F32)
    l1b = din("l1b", [128, DC], F32)
    l2g = din("l2g", [128, DC], F32)
    l2b = din("l2b", [128, DC], F32)
    vmask = din("vmask", [128, NKC], F32)
    epsc = din("epsc", [128, 1], F32)
    rowmask = din("rowmask", [128, M], F32)

    hT = nc.dram_tensor("hT", [D, M], F32, kind="ExternalOutput").ap()

    xT_r = xT.rearrange("(c p) n -> p c n", p=128)
    xTq_r = xTq.rearrange("(c p) n -> p c n", p=128)
    wq_r = wq.rearrange("(c p) n -> p c n", p=128)
    wk_r = wk.rearrange("(c p) n -> p c n", p=128)
    wv_r = wv.rearrange("(c p) n -> p c n", p=128)
    wproj_r = wproj.rearrange("(c p) n -> p c n", p=128)
    wfc_r = wfc.rearrange("(c p) n -> p c n", p=128)
    hT_r = hT.rearrange("(c p) n -> c p n", p=128)

    with tile.TileContext(nc) as tc, ExitStack() as ctx:
        const = ctx.enter_context(tc.tile_pool(name="const", bufs=1))

        def cload(name, shape, dt, src):
            t = const.tile(shape, dt, tag=name, name=name + "_t")
            nc.sync.dma_start(out=t, in_=src)
            return t

        consts = {}

        def cloads():
            consts["onesr"] = cload("onesr", [1, M], F32R, onesr)
            consts["ones128"] = cload("ones128", [128, 128], F32R, ones128)
            consts["vmask"] = cload("vmask", [128, NKC], F32, vmask)
            consts["epsc"] = cload("epsc", [128, 1], F32, epsc)
            consts["rowmask"] = cload("rowmask", [128, M], F32, rowmask)
            consts["bprj"] = cload("bprj", [128, DC], F32, bprj)
            consts["bfc"] = cload("bfc", [128, FC], F32, bfc)
            consts["bout"] = cload("bout", [128, DC], F32, bout)
            consts["l1g"] = cload("l1g", [128, DC], F32, l1g)
            consts["l1b"] = cload("l1b", [128, DC], F32, l1b)
            consts["l2g"] = cload("l2g", [128, DC], F32, l2g)
            consts["l2b"] = cload("l2b", [128, DC], F32, l2b)

        # ---------------- LN helpers (transposed layout) ----------------
        def ln_stats_bc(lnp_t, lnp_k, psum_sum, psum_ssq):
            m_bc = lnp_k.tile([128, M], F32, tag="mbc", name="mbc")
            nc.vector.tensor_scalar_mul(m_bc, psum_sum, 1.0 / D)
            mm = lnp_t.tile([128, M], F32, tag="mm", name="mm")
            nc.vector.tensor_mul(mm, m_bc, m_bc)
            var = lnp_t.tile([128, M], F32, tag="var", name="var")
            nc.vector.scalar_tensor_tensor(var, psum_ssq, 1.0 / D, mm,
                                           op0=OP.mult, op1=OP.subtract)
            sd = lnp_t.tile([128, M], F32, tag="sd", name="sd")
            nc.scalar.activation(sd, var, AF.Sqrt, bias=consts["epsc"][:, 0:1])
            rstd = lnp_k.tile([128, M], F32, tag="rstd", name="rstd")
            nc.vector.reciprocal(rstd, sd)
            return m_bc, sd, rstd

        def ln_apply_chunks(lns, y_t, m_bc, rstd, g_t, b_t, out_cb):
            for c2 in range(DC):
                t1 = lns.tile([128, M], F32, tag="t1", name="t1")
                nc.vector.tensor_sub(t1, y_t[:, c2, :].bitcast(F32), m_bc)
                t2 = lns.tile([128, M], F32, tag="t2", name="t2")
                nc.vector.tensor_mul(t2, t1, rstd)
                t3 = lns.tile([128, M], F32, tag="t3", name="t3")
                nc.vector.tensor_scalar(t3, t2, g_t[:, c2:c2 + 1],
                                        b_t[:, c2:c2 + 1],
                                        op0=OP.mult, op1=OP.add)
                out_cb(c2, t3)

        def ln_apply(y_t, psum_sum, psum_ssq, g_t, b_t, out_cb):
            with tc.tile_pool(name="lnp", bufs=1) as lnp, \
                 tc.tile_pool(name="lns", bufs=2) as lns:
                m_bc, sd, rstd = ln_stats_bc(lnp, lnp, psum_sum, psum_ssq)
                ln_apply_chunks(lns, y_t, m_bc, rstd, g_t, b_t, out_cb)

        big = ctx.enter_context(tc.tile_pool(name="big", bufs=1))
        for _rep in range(reps):
            kT_t = big.tile([128, DC, T], F32R, tag="slotL", name="kT")
            vP_t = big.tile([128, NKC, VW], F32R, tag="slotM", name="vP")
            qT_t = big.tile([128, DC, M], F32R, tag="slotS", name="qT")

            # -------- Merged phase: QKV projections + attention --------
            with tc.tile_pool(name="uaccp", bufs=1) as uaccp:
                uacc = uaccp.tile([DH + 1, H, M], F32R, tag="uacc", name="uacc")

                # Q^T [768, 512] (scoped pool, freed before the tb loop)
                with tc.tile_pool(name="qsc", bufs=1) as qsc, \
                     tc.tile_pool(name="qsc2", bufs=2) as qsc2, \
                     tc.tile_pool(name="psQ", bufs=2, space="PSUM") as psQ:
                    xTq_t = qsc.tile([128, DC, M], F32R, tag="xTq", name="xTq_t")
                    for dc in range(DC):
                        nc.sync.dma_start(out=xTq_t[:, dc, :], in_=xTq_r[:, dc, :])
                    bq_t = cload("bq_pc", [128, DC], F32, bq_pc)
                    bk_t = cload("bk_pc", [128, DC], F32, bk_pc)
                    for qc in range(DC):
                        wq_c = qsc2.tile([128, DC, 128], F32R, tag="wqs",
                                         name="wq_c")
                        for dc in range(DC):
                            nc.sync.dma_start(
                                out=wq_c[:, dc, :],
                                in_=wq_r[:, dc, qc * 128:(qc + 1) * 128])
                        pq = psQ.tile([128, M], F32, tag="pq", name="pq")
                        for dc in range(DC):
                            nc.tensor.matmul(pq, wq_c[:, dc, :], xTq_t[:, dc, :],
                                             start=(dc == 0), stop=(dc == DC - 1))
                        nc.vector.tensor_scalar_add(qT_t[:, qc, :], pq,
                                                    bq_t[:, qc:qc + 1])

                cloads()
                # K^T / V' / attention partials per token-block
                aT_t = big.tile([128, DC, M], F32R, tag="slotA", name="aT")
                with tc.tile_pool(name="p1", bufs=1) as p1, \
                     tc.tile_pool(name="p1s", bufs=3) as p1s, \
                     tc.tile_pool(name="p1x", bufs=1) as p1x, \
                     tc.tile_pool(name="attp", bufs=3) as attp, \
                     tc.tile_pool(name="psK", bufs=2, space="PSUM") as psK, \
                     tc.tile_pool(name="psV", bufs=4, space="PSUM") as psV, \
                     tc.tile_pool(name="atts", bufs=2) as atts, \
                     tc.tile_pool(name="psS", bufs=2, space="PSUM") as psS, \
                     tc.tile_pool(name="psA", bufs=1, space="PSUM") as psA, \
                     tc.tile_pool(name="psB", bufs=1, space="PSUM") as psB:
                    bv_t = p1.tile([1, VW], F32R, tag="bv", name="bv_t")
                    nc.sync.dma_start(out=bv_t, in_=bv)
                    wv_t = p1.tile([128, DC, VW], F32R, tag="wv", name="wv_t")
                    for dc in range(DC):
                        nc.sync.dma_start(out=wv_t[:, dc, :], in_=wv_r[:, dc, :])

                    for tb in range(4):
                        xtb = p1x.tile([128, DC, M], F32R, tag="xtb", name="xtb")
                        for dc in range(DC):
                            nc.sync.dma_start(out=xtb[:, dc, :],
                                              in_=xT_r[:, dc, tb * M:(tb + 1) * M])
                        for kc in range(DC):
                            wk_c = p1s.tile([128, DC, 128], F32R, tag="wks",
                                            name="wk_c")
                            for dc in range(DC):
                                nc.sync.dma_start(
                                    out=wk_c[:, dc, :],
                                    in_=wk_r[:, dc, kc * 128:(kc + 1) * 128])
                            pk = psK.tile([128, M], F32, tag="pk", name="pk")
                            for dc in range(DC):
                                nc.tensor.matmul(pk, wk_c[:, dc, :], xtb[:, dc, :],
                                                 start=(dc == 0),
                                                 stop=(dc == DC - 1))
                            nc.vector.tensor_scalar_add(
                                kT_t[:, kc, tb * M:(tb + 1) * M], pk,
                                bk_t[:, kc:kc + 1])
                        for tq in range(4):
                            tci = tb * 4 + tq
                            for vb in range(2):
                                pv = psV.tile([128, VW // 2], F32, tag="pv",
                                              name="pv")
                                for dc in range(DC):
                                    nc.tensor.matmul(
                                        pv, xtb[:, dc, tq * 128:(tq + 1) * 128],
                                        wv_t[:, dc,
                                             vb * (VW // 2):(vb + 1) * (VW // 2)],
                                        start=(dc == 0), stop=False)
                                nc.tensor.matmul(
                                    pv, consts["onesr"][0:1, 0:128],
                                    bv_t[0:1, vb * (VW // 2):(vb + 1) * (VW // 2)],
                                    start=False, stop=True)
                                nc.vector.tensor_scalar_mul(
                                    vP_t[:, tci,
                                         vb * (VW // 2):(vb + 1) * (VW // 2)],
                                    pv, consts["vmask"][:, tci:tci + 1])
                        # attention partials over this tb's 4 key chunks
                        for h in range(H):
                            po = (h % 2) * 64
                            chk = h // 2
                            pu = psA.tile([DH + 1, M], F32, tag="pu", name="pu")
                            for kci in range(4):
                                kc = tb * 4 + kci
                                s = psS.tile([128, M], F32, tag="s", name="s")
                                nc.tensor.matmul(
                                    s,
                                    kT_t[po:po + 64, chk, kc * 128:(kc + 1) * 128],
                                    qT_t[po:po + 64, chk, :],
                                    start=True, stop=True)
                                e = attp.tile([128, M], F32R, tag="exp", name="e")
                                nc.scalar.activation(e, s, AF.Exp)
                                nc.tensor.matmul(
                                    pu,
                                    vP_t[:, kc, h * (DH + 1):(h + 1) * (DH + 1)],
                                    e, start=(kci == 0), stop=(kci == 3))
                            if tb == 0:
                                nc.vector.tensor_copy(uacc[:, h, :], pu)
                            elif tb < 3:
                                nc.vector.tensor_add(uacc[:, h, :],
                                                     uacc[:, h, :], pu)
                            else:
                                # final partial: finish softmax for this head
                                nc.vector.tensor_add(uacc[:, h, :],
                                                     uacc[:, h, :], pu)
                                srow = atts.tile([1, M], F32R, tag="srow",
                                                 name="srow")
                                nc.vector.tensor_copy(srow,
                                                      uacc[DH:DH + 1, h, :])
                                pb = psB.tile([64, M], F32, tag="pb", name="pb")
                                nc.tensor.matmul(pb, consts["onesr"][0:1, 0:64],
                                                 srow, start=True, stop=True)
                                rinv = atts.tile([64, M], F32, tag="rinv",
                                                 name="rinv")
                                nc.vector.reciprocal(rinv, pb)
                                nc.vector.tensor_mul(
                                    aT_t[po:po + 64, chk, :],
                                    uacc[0:DH, h, :].bitcast(F32), rinv)

            # ------------ Phase 3: proj + residual + LN1 ------------
            with tc.tile_pool(name="foldp", bufs=1) as foldp:
                    nT_t = big.tile([128, DC, M], F32R, tag="slotS", name="nT")
                    with tc.tile_pool(name="p3", bufs=1) as p3, \
                         tc.tile_pool(name="p3s", bufs=2) as p3s, \
                         tc.tile_pool(name="psP", bufs=2, space="PSUM") as psP, \
                         tc.tile_pool(name="psT", bufs=1, space="PSUM") as psT:
                        wproj_t = p3.tile([128, DC, D], F32R, tag="wproj",
                                          name="wproj_t")
                        nc.sync.dma_start(out=wproj_t, in_=wproj_r)
                        y1_t = big.tile([128, DC, M], F32R, tag="slotL",
                                        name="y1")
                        psum_sum = psT.tile([128, M], F32, tag="s1",
                                            name="psum_sum")
                        psum_ssq = psT.tile([128, M], F32, tag="s2",
                                            name="psum_ssq")
                        for do in range(DC):
                            pp = psP.tile([128, M], F32, tag="pp", name="pp")
                            for di in range(DC):
                                nc.tensor.matmul(
                                    pp, wproj_t[:, di, do * 128:(do + 1) * 128],
                                    aT_t[:, di, :], start=(di == 0),
                                    stop=(di == DC - 1))
                            xr = p3s.tile([128, M], F32R, tag="xr", name="xr")
                            nc.sync.dma_start(out=xr, in_=xTq_r[:, do, :])
                            nc.vector.scalar_tensor_tensor(
                                y1_t[:, do, :], pp, consts["bprj"][:, do:do + 1],
                                xr.bitcast(F32), op0=OP.add, op1=OP.add)
                            sq = p3s.tile([128, M], F32R, tag="sq", name="sq")
                            nc.vector.tensor_mul(sq, y1_t[:, do, :].bitcast(F32),
                                                 y1_t[:, do, :].bitcast(F32))
                            nc.tensor.matmul(psum_sum, consts["ones128"], y1_t[:, do, :],
                                             start=(do == 0), stop=(do == DC - 1))
                            nc.tensor.matmul(psum_ssq, consts["ones128"], sq,
                                             start=(do == 0), stop=(do == DC - 1))

                        ln1p = p3  # reuse phase-3 resident pool for LN1 stats
                        m_bc, sd_bc, rstd_bc = ln_stats_bc(ln1p, psum_sum,
                                                           psum_ssq)
                        m_row = foldp.tile([1, M], F32R, tag="mrow",
                                           name="m_row")
                        nc.vector.tensor_copy(m_row, m_bc[0:1, :])
                        isd_row = foldp.tile([1, M], F32R, tag="isdrow",
                                             name="isd_row")
                        nc.vector.tensor_copy(isd_row, sd_bc[0:1, :])
                        rm_bc = foldp.tile([128, M], F32, tag="rmbc",
                                           name="rm_bc")
                        nc.vector.tensor_mul(rm_bc, rstd_bc, consts["rowmask"])

                        def to_nT(c2, t3):
                            nc.vector.tensor_mul(nT_t[:, c2, :], t3,
                                                 consts["rowmask"])

                        with tc.tile_pool(name="lns1", bufs=2) as lns1:
                            ln_apply_chunks(lns1, y1_t, m_bc, rstd_bc,
                                            consts["l1g"], consts["l1b"], to_nT)

                    # ------------ Phase 4: FFN + residual + LN2 ------------
                    wt1_t = foldp.tile([1, FF], F32R, tag="wt1", name="wt1_t")
                    nc.sync.dma_start(out=wt1_t, in_=wt1)
                    wt2_t = foldp.tile([1, FF], F32R, tag="wt2", name="wt2_t")
                    nc.sync.dma_start(out=wt2_t, in_=wt2)
                    with tc.tile_pool(name="p4a", bufs=2) as p4a, \
                         tc.tile_pool(name="p4h", bufs=2) as p4h, \
                         tc.tile_pool(name="psM", bufs=1, space="PSUM") as psM:
                        psm = [psM.tile([128, M], F32, tag=f"m{do}",
                                        name=f"psm{do}") for do in range(DC)]
                        with tc.tile_pool(name="p4w", bufs=3) as p4w, \
                             tc.tile_pool(name="psF", bufs=2, space="PSUM") as psF:
                            for f in range(FC):
                                wfcf = p4w.tile([128, DC, 128], F32R, tag="wfcf",
                                                name="wfcf")
                                nc.sync.dma_start(
                                    out=wfcf,
                                    in_=wfc_r[:, :, f * 128:(f + 1) * 128])
                                woutf = p4w.tile([128, D], F32R, tag="woutf",
                                                 name="woutf")
                                nc.sync.dma_start(
                                    out=woutf, in_=wout[f * 128:(f + 1) * 128, :])
                                pf = psF.tile([128, M], F32, tag="pf", name="pf")
                                for dc in range(DC):
                                    nc.tensor.matmul(pf, wfcf[:, dc, :],
                                                     y1_t[:, dc, :],
                                                     start=(dc == 0), stop=False)
                                nc.tensor.matmul(
                                    pf, wt1_t[0:1, f * 128:(f + 1) * 128],
                                    m_row, start=False, stop=False)
                                nc.tensor.matmul(
                                    pf, wt2_t[0:1, f * 128:(f + 1) * 128],
                                    isd_row, start=False, stop=True)
                                nc.vector.tensor_mul(pf, pf, rm_bc)
                                a1 = p4a.tile([128, M], F32R, tag="a1", name="a1")
                                nc.scalar.activation(a1, pf, AF.Gelu_apprx_tanh,
                                                     bias=consts["bfc"][:, f:f + 1])
                                for do in range(DC):
                                    nc.tensor.matmul(
                                        psm[do],
                                        woutf[:, do * 128:(do + 1) * 128],
                                        a1, start=(f == 0), stop=(f == FC - 1))

                        with tc.tile_pool(name="psT2", bufs=1,
                                          space="PSUM") as psT2:
                            y2_t = big.tile([128, DC, M], F32R,
                                            tag="slotM", name="y2")
                            psum_sum2 = psT2.tile([128, M], F32, tag="s1",
                                                  name="psum_sum2")
                            psum_ssq2 = psT2.tile([128, M], F32, tag="s2",
                                                  name="psum_ssq2")
                            for do in range(DC):
                                nc.vector.scalar_tensor_tensor(
                                    y2_t[:, do, :], psm[do],
                                    consts["bout"][:, do:do + 1],
                                    nT_t[:, do, :].bitcast(F32),
                                    op0=OP.add, op1=OP.add)
                                sq = p4a.tile([128, M], F32R, tag="sq2",
                                              name="sq2")
                                nc.vector.tensor_mul(
                                    sq, y2_t[:, do, :].bitcast(F32),
                                    y2_t[:, do, :].bitcast(F32))
                                nc.tensor.matmul(psum_sum2, consts["ones128"],
                                                 y2_t[:, do, :],
                                                 start=(do == 0),
                                                 stop=(do == DC - 1))
                                nc.tensor.matmul(psum_ssq2, consts["ones128"], sq,
                                                 start=(do == 0),
                                                 stop=(do == DC - 1))

                            def to_h(c2, t3):
                                hc = p4h.tile([128, M], F32, tag="hc", name="hc")
                                nc.vector.tensor_mul(hc, t3, consts["rowmask"])
                                nc.sync.dma_start(out=hT_r[c2], in_=hc)

                            ln_apply(y2_t, psum_sum2, psum_ssq2, consts["l2g"], consts["l2b"],
                                     to_h)

    nc.compile()
    return nc


def _shared_arrays(inputs):
    f32 = np.float32
    w_qkv = np.ascontiguousarray(inputs["w_qkv"], dtype=f32)
    b_qkv = np.ascontiguousarray(inputs["b_qkv"], dtype=f32)

    def pc(v):  # [C*128] -> [128, C] column-chunk layout
        v = np.ascontiguousarray(v, dtype=f32)
        return np.ascontiguousarray(v.reshape(-1, 128).T)

    w_fc_raw = np.ascontiguousarray(inputs["w_fc"], dtype=np.float64)
    wfcg = (w_fc_raw * np.asarray(inputs["ln1_g"],
                                  dtype=np.float64)[:, None]).astype(f32)
    wv_ext = np.zeros((D, VW), f32)
    bv_ext = np.zeros((1, VW), f32)
    for h in range(H):
        wv_ext[:, h * (DH + 1):h * (DH + 1) + DH] = \
            w_qkv[:, 2 * D + h * DH:2 * D + (h + 1) * DH]
        bv_ext[0, h * (DH + 1):h * (DH + 1) + DH] = \
            b_qkv[2 * D + h * DH:2 * D + (h + 1) * DH]
        bv_ext[0, h * (DH + 1) + DH] = 1.0

    return dict(
        wq=np.ascontiguousarray(w_qkv[:, 0:D]),
        bq_pc=pc(b_qkv[0:D]),
        wk=np.ascontiguousarray(w_qkv[:, D:2 * D]),
        bk_pc=pc(b_qkv[D:2 * D]),
        wv=wv_ext,
        bv=bv_ext,
        wproj=np.ascontiguousarray(inputs["w_proj"], dtype=f32),
        wfc=wfcg,
        wt1=(-wfcg.sum(axis=0, dtype=np.float64)).astype(f32)[None, :],
        wt2=(w_fc_raw * np.asarray(inputs["ln1_b"], dtype=np.float64)[:, None]
             ).sum(axis=0).astype(f32)[None, :],
        wout=np.ascontiguousarray(inputs["w_out"], dtype=f32),
        onesr=np.ones((1, M), f32),
        epsc=np.full((128, 1), EPS, f32),
        ones128=np.ones((128, 128), f32),
        bprj=pc(inputs["b_proj"]),
        bfc=pc(inputs["b_fc"]),
        bout=pc(inputs["b_out"]),
        l1g=pc(inputs["ln1_g"]),
        l1b=pc(inputs["ln1_b"]),
        l2g=pc(inputs["ln2_g"]),
        l2b=pc(inputs["ln2_b"]),
    )


def make_in_maps(inputs):
    inputs = {k: np.asarray(v) for k, v in inputs.items()}
    x = np.ascontiguousarray(inputs["x"], dtype=np.float32)
    lengths = np.asarray(inputs["lengths"]).astype(np.int64)
    shared = _shared_arrays(inputs)
    pos = np.arange(T)
    in_maps = []
    for c in range(8):
        b, r = divmod(c, 4)
        sl = slice(r * M, (r + 1) * M)
        xTb = np.ascontiguousarray(x[b].T)
        km = (pos < lengths[b]).astype(np.float32)
        rm = (pos[sl] < lengths[b]).astype(np.float32)
        m = dict(shared)
        m["xT"] = xTb
        m["xTq"] = np.ascontiguousarray(xTb[:, sl])
        m["vmask"] = np.ascontiguousarray(km.reshape(NKC, 128).T)
        m["rowmask"] = np.ascontiguousarray(np.broadcast_to(rm[None, :], (128, M)))
        in_maps.append(m)
    return in_maps


def get_program(reps=1):
    key = f"nc{reps}"
    if key not in _STATE:
        _STATE[key] = _build_program(reps)
    return _STATE[key]


def kernel(**inputs) -> np.ndarray:
    from concourse.bass_utils import run_bass_kernel_spmd

    nc = get_program()
    in_maps = make_in_maps(inputs)
    res = run_bass_kernel_spmd(nc, in_maps, list(range(8)), trace=False)
    out = np.zeros((B, T, D), np.float32)
    for c in range(8):
        b, r = divmod(c, 4)
        out[b, r * M:(r + 1) * M, :] = res.results[c]["hT"].T
    return out
